# revision 18
# baseline (speedup 1.0000x reference)
"""Trainium2 kernel for: out = tanh(x @ scatter_nd(nonzero_ind, kernel_vector, (20000, 4096)) + bias).

Strategy (8 NeuronCores), W-resident / x-streaming, units sharded x8:
  core c owns W[:, c*512:(c+1)*512] (20096 x 512 fp16, SBUF-resident) and
  computes out[:, c*512:(c+1)*512] = x @ W_c for the full batch.

v3 (trace-driven, vs 569us baseline):
  - ALL DMA traffic rides the two HWDGE rings (sync/scalar), ~115 GB/s
    each during chunk 0 -- the SWDGE (gpsimd) queue degrades to ~70 GB/s
    under fabric load (descriptor rings live in SBUF and contend).
  - W is split into k-tile pairs and each W pair is issued on the same
    engine right after its x pair: ring FIFO = [x_p, W_p, x_p+2, ...],
    so W arrives just-in-time with its x pair and can never front-run
    the x stream (engine streams run ahead of the PE, so issue-side
    "pacing" alone does nothing -- learned the hard way).
  - Next chunk's first x pairs are issued BEFORE the drain casts so they
    don't queue behind cast sem-waits on the same engine streams.
  - PE warmup: memset + 8 garbage matmuls cover the first-DMA wait and
    the HAM cold-clock window.
  - Chunks [1024, 512, 512]: PSUM pool A holds s=0 banks, pool B s=1;
    512-chunks alternate A/B so each boundary only waits the first bank
    set, casts split across Vector AND Scalar engines, one consolidated
    stage tile + out-DMA halves per chunk -> ~3.5us tail.
"""

import numpy as np

P = 128
B, K, U = 2048, 20000, 4096
USPLIT = 8
KT = 157                 # k-tiles (full contraction per core)
KTP = 158                # padded to even for x k-tile pairs
KPAD_X = KTP * P         # 20224 rows (224 zero pad) for x packing
U_SH = U // USPLIT       # 512 unit cols per core
NUS = U_SH // P          # 4 W subtiles (stationary blocks) per k-tile
NXP = KTP // 2           # 79 k-tile pairs (x and W)
XCOLS = 2048             # uniform x pool tile cols (max chunk: 2*1024)

# chunk config: list of (batch_size, n_batch_blocks); BBLK = size // nbb = 512
CHUNKS = [(1024, 2), (512, 1), (512, 1)]
BBLK = 512

TRACE = False            # set by test harness for profiled runs
LAST_RESULT = None       # BassKernelResults of the last run (for the harness)

_NC_CACHE = {}


def _build_nc():
    from concourse import bacc
    import concourse.mybir as mybir
    import concourse.tile as tile

    f32 = mybir.dt.float32
    f16 = mybir.dt.float16
    bf16 = mybir.dt.bfloat16

    nc = bacc.Bacc("TRN2", target_bir_lowering=False, debug=False)

    # x^T k-tile pairs per chunk: xt{ch}[pair, p, j*bch + b]
    #   = x[b0(ch) + b, (2*pair+j)*128 + p]  (fp16, zero-padded rows)
    xt_d = [
        nc.dram_tensor(f"xt{ch}", [NXP, P, 2 * bch], f16, kind="ExternalInput").ap()
        for ch, (bch, nbb) in enumerate(CHUNKS)
    ]
    # W k-tile pairs: w[pair, p, j*512 + u] = W[(2*pair+j)*128 + p, u]
    w_d = nc.dram_tensor("w_sh", [NXP, P, 2 * U_SH], f16, kind="ExternalInput").ap()
    # out per chunk: o{ch}[p, us*nbb*BBLK + s*BBLK + b] = z^T[us*128+p, ...]
    o_d = [
        nc.dram_tensor(f"o{ch}", [P, NUS, nbb * BBLK], bf16,
                       kind="ExternalOutput").ap()
        for ch, (bch, nbb) in enumerate(CHUNKS)
    ]

    with tile.TileContext(nc) as tc:
        with (
            tc.tile_pool(name="resid", bufs=1) as respool,
            tc.tile_pool(name="xpool", bufs=6) as xpool,
            tc.tile_pool(name="stage", bufs=1) as spool,
            tc.tile_pool(name="warm", bufs=1) as wmpool,
            tc.tile_pool(name="psumA", bufs=1, space="PSUM") as psumA,
            tc.tile_pool(name="psumB", bufs=1, space="PSUM") as psumB,
        ):
            xq = [nc.sync, nc.scalar]
            prefetched = {}

            wres = [
                respool.tile([P, 2 * U_SH], f16, tag=f"w{p}", name=f"w{p}")
                for p in range(NXP)
            ]

            def x_issue(ch, pair):
                t = xpool.tile([P, XCOLS], f16, tag="xs", name="xs")
                bch = CHUNKS[ch][0]
                eng = xq[pair % 2]
                eng.dma_start(t[:, :2 * bch], xt_d[ch][pair])
                if ch == 0:
                    # W pair rides right behind its x pair on the same ring
                    eng.dma_start(wres[pair][:], w_d[pair])
                prefetched[(ch, pair)] = t

            # --- PE warmup: keep the PE busy through the HAM cold window
            # while the first x/W DMAs are in flight.  Garbage matmuls into
            # the slot chunk 0 will overwrite (start=True clears the bank).
            wtile = wmpool.tile([P, BBLK], f16, tag="wm", name="wm")
            nc.vector.memset(wtile[:], 0.0)
            wps = psumA.tile([P, BBLK], f32, tag="pa0", name="wps")

            # x+W pairs 0/1 first on each ring (ring FIFO: first issued =
            # first completed).
            x_issue(0, 0)
            x_issue(0, 1)

            for _ in range(7):
                nc.tensor.matmul(wps[:], wtile[:, 0:P], wtile[:],
                                 start=True, stop=True)

            n512 = 0
            for ch, (bch, nbb) in enumerate(CHUNKS):
                # PSUM: s=0 blocks in pool A, s=1 in pool B (nbb=2);
                # nbb=1 chunks alternate pools A, B, A, ...
                if nbb == 2:
                    pools = [(psumA, "a"), (psumB, "b")]
                else:
                    pools = [[(psumA, "a"), (psumB, "b")][n512 % 2]]
                    n512 += 1
                psums = [
                    [pool.tile([P, BBLK], f32, tag=f"p{pc}{us}",
                               name=f"ps{ch}_{us}_{s}")
                     for s, (pool, pc) in enumerate(pools)]
                    for us in range(NUS)
                ]

                for pair in range(NXP):
                    if (ch, pair) not in prefetched:
                        x_issue(ch, pair)
                    xs = prefetched.pop((ch, pair))
                    for j in range(2):
                        kt = 2 * pair + j
                        if kt >= KT:
                            break
                        # first k-tile after a boundary: touch the banks in
                        # the order the previous chunk's casts free them
                        us_order = [0, 2, 1, 3] if (kt == 0 and ch > 0) \
                            else range(NUS)
                        for us in us_order:
                            for s in range(nbb):
                                nc.tensor.matmul(
                                    psums[us][s][:],
                                    wres[pair][:, j * U_SH + us * P:
                                               j * U_SH + (us + 1) * P],
                                    xs[:, j * bch + s * BBLK:
                                           j * bch + (s + 1) * BBLK],
                                    start=(kt == 0),
                                    stop=(kt == KT - 1),
                                )

                # Prefetch next chunk's first x pairs BEFORE the drain --
                # the casts below block the engine streams on matmul sems.
                if ch + 1 < len(CHUNKS):
                    for pp in range(3):
                        x_issue(ch + 1, pp)

                # Drain: cast PSUM -> one stage tile; vector does us 0-1,
                # scalar does us 2-3 in parallel; pool-A banks (which the
                # next chunk needs first) are cast before pool-B banks.
                st = spool.tile([P, NUS * nbb * BBLK], bf16,
                                tag=f"st{ch}", name=f"st{ch}")
                for s in range(nbb):          # s=0 (pool A) first
                    for us in range(NUS):
                        dst = st[:, (us * nbb + s) * BBLK:
                                 (us * nbb + s + 1) * BBLK]
                        if us < 2:
                            nc.vector.tensor_copy(dst, psums[us][s][:])
                        else:
                            nc.scalar.copy(dst, psums[us][s][:])
                # out-DMA halves: us 0-1 on sync after vector's casts,
                # us 2-3 on scalar after its own casts.
                half = 2 * nbb * BBLK
                nc.sync.dma_start(o_d[ch][:, 0:2], st[:, :half])
                nc.scalar.dma_start(o_d[ch][:, 2:4], st[:, half:])

    nc.compile()
    return nc


def _get_nc():
    if "nc" not in _NC_CACHE:
        _NC_CACHE["nc"] = _build_nc()
    return _NC_CACHE["nc"]


def kernel(x, kernel_vector, bias, nonzero_ind):
    global LAST_RESULT
    from concourse.bass_utils import run_bass_kernel_spmd

    x = np.asarray(x, dtype=np.float32)
    kernel_vector = np.asarray(kernel_vector, dtype=np.float32)
    bias = np.asarray(bias, dtype=np.float32)
    nonzero_ind = np.asarray(nonzero_ind)

    nc = _get_nc()

    # Host scatter: dense weights [KPAD_X, U] fp16 (rows >= 20000 stay zero).
    rows = nonzero_ind[:, 0].astype(np.int64)
    cols = nonzero_ind[:, 1].astype(np.int64)
    w_full = np.zeros(KPAD_X * U, np.float32)
    np.add.at(w_full, rows * U + cols, kernel_vector)
    w_full = w_full.reshape(KPAD_X, U).astype(np.float16)

    # x^T padded to KPAD_X rows, fp16.
    x16 = x.astype(np.float16)
    xt = np.zeros((KPAD_X, B), np.float16)
    xt[:K] = x16.T

    # Per-chunk pair layout: xt{ch}[pair, p, j*bch + b] = xt[(2p+j)*128+p, b0+b]
    xt_chunks = []
    b0 = 0
    for bch, nbb in CHUNKS:
        xc = xt[:, b0:b0 + bch].reshape(NXP, 2, P, bch).transpose(0, 2, 1, 3)
        xt_chunks.append(np.ascontiguousarray(xc.reshape(NXP, P, 2 * bch)))
        b0 += bch

    in_maps = []
    for c in range(USPLIT):
        # W pairs: w[pair, p, j*512 + u] = W[(2*pair+j)*128 + p, c*512 + u]
        w_sh = w_full[:, c * U_SH:(c + 1) * U_SH]
        w_sh = w_sh.reshape(NXP, 2, P, U_SH).transpose(0, 2, 1, 3)
        w_sh = np.ascontiguousarray(w_sh.reshape(NXP, P, 2 * U_SH))
        m = {"w_sh": w_sh}
        for ch in range(len(CHUNKS)):
            m[f"xt{ch}"] = xt_chunks[ch]
        in_maps.append(m)

    kwargs = {}
    if TRACE:
        kwargs = dict(trace=True, trace_cores=list(range(8)))
    res = run_bass_kernel_spmd(nc, in_maps, core_ids=list(range(8)), **kwargs)
    LAST_RESULT = res

    out = np.empty((B, U), np.float32)
    for c in range(USPLIT):
        b0 = 0
        for ch, (bch, nbb) in enumerate(CHUNKS):
            # [P, NUS, nbb*BBLK] -> [nbb, BBLK, NUS, P] -> [bch, U_SH]
            blk = (
                res.results[c][f"o{ch}"]
                .astype(np.float32)
                .reshape(P, NUS, nbb, BBLK)
                .transpose(2, 3, 1, 0)
                .reshape(bch, U_SH)
            )
            out[b0:b0 + bch, c * U_SH:(c + 1) * U_SH] = blk
            b0 += bch
    out += bias[None, :]
    np.tanh(out, out=out)
    return out


# revision 19
# speedup vs baseline: 1.0194x; 1.0194x over previous
"""Trainium2 kernel for: out = tanh(x @ scatter_nd(nonzero_ind, kernel_vector, (20000, 4096)) + bias).

Strategy (8 NeuronCores), W-resident / x-streaming, units sharded x8:
  core c owns W[:, c*512:(c+1)*512] (20096 x 512 fp16, SBUF-resident) and
  computes out[:, c*512:(c+1)*512] = x @ W_c for the full batch.

v5 (trace-driven, vs 569us baseline):
  - Every dma_start issue WAITS on its completion-semaphore lane (Tile
    round-robins 4 lanes per engine) and blocks the engine stream until
    the lane's previous DMA completed (~2-4us incl. HBM receipt).  So
    few/large transfers win: x rides as 1 MB 4-k-tile quads, W as 512 KB
    quads -- 2 DMAs per ring per 13.8us cycle, far under lane capacity.
  - W quad q is issued on the same engine right after x quad q: ring
    FIFO = [x_q, W_q, x_q+2, ...] -> W arrives just-in-time and can
    never front-run the x stream (engine streams run ahead of the PE,
    so issue-side "pacing" alone does nothing).
  - Both HWDGE rings (sync/scalar) carry the streams (~115 GB/s each in
    chunk 0); out-DMAs of non-final chunks go to the otherwise idle
    gpsimd SWDGE queue so they never block the x issue streams.
  - PE warmup: memset + 7 garbage matmuls cover the first-DMA wait and
    the HAM cold-clock window.
  - Chunks [1024, 512, 512]: PSUM pool A holds s=0 banks, pool B s=1;
    512-chunks alternate A/B; next chunk's first x quads are issued
    BEFORE the drain casts (which block the engine streams); casts are
    split across Vector AND Scalar, A-banks first; ~3.5us tail.
"""

import numpy as np

P = 128
B, K, U = 2048, 20000, 4096
USPLIT = 8
KT = 157                 # k-tiles (full contraction per core)
NXQ = 40                 # 4-k-tile quads (kt 157-159 zero pad)
KPAD = NXQ * 4 * P       # 20480 padded rows
U_SH = U // USPLIT       # 512 unit cols per core
NUS = U_SH // P          # 4 W subtiles (stationary blocks) per k-tile
XCOLS = 4096             # uniform x pool tile cols (max chunk: 4*1024)

# chunk config: list of (batch_size, n_batch_blocks); BBLK = size // nbb = 512
CHUNKS = [(1024, 2), (512, 1), (512, 1)]
BBLK = 512

TRACE = False            # set by test harness for profiled runs
LAST_RESULT = None       # BassKernelResults of the last run (for the harness)

_NC_CACHE = {}


def _build_nc():
    from concourse import bacc
    import concourse.mybir as mybir
    import concourse.tile as tile

    f32 = mybir.dt.float32
    f16 = mybir.dt.float16
    bf16 = mybir.dt.bfloat16

    nc = bacc.Bacc("TRN2", target_bir_lowering=False, debug=False)

    # x^T quads per chunk: xt{ch}[q, p, j*bch + b]
    #   = x[b0(ch) + b, (4q+j)*128 + p]  (fp16, zero-padded rows)
    xt_d = [
        nc.dram_tensor(f"xt{ch}", [NXQ, P, 4 * bch], f16, kind="ExternalInput").ap()
        for ch, (bch, nbb) in enumerate(CHUNKS)
    ]
    # W quads: w[q, p, j*512 + u] = W[(4q+j)*128 + p, u]
    w_d = nc.dram_tensor("w_sh", [NXQ, P, 4 * U_SH], f16, kind="ExternalInput").ap()
    # out per chunk: o{ch}[p, us*nbb*BBLK + s*BBLK + b] = z^T[us*128+p, ...]
    o_d = [
        nc.dram_tensor(f"o{ch}", [P, NUS, nbb * BBLK], bf16,
                       kind="ExternalOutput").ap()
        for ch, (bch, nbb) in enumerate(CHUNKS)
    ]

    with tile.TileContext(nc) as tc:
        with (
            tc.tile_pool(name="resid", bufs=1) as respool,
            tc.tile_pool(name="xpool", bufs=4) as xpool,
            tc.tile_pool(name="stage", bufs=1) as spool,
            tc.tile_pool(name="warm", bufs=1) as wmpool,
            tc.tile_pool(name="psumA", bufs=1, space="PSUM") as psumA,
            tc.tile_pool(name="psumB", bufs=1, space="PSUM") as psumB,
        ):
            xq = [nc.sync, nc.scalar]
            prefetched = {}

            wres = [
                respool.tile([P, 4 * U_SH], f16, tag=f"w{q}", name=f"w{q}")
                for q in range(NXQ)
            ]

            def x_issue(ch, q):
                t = xpool.tile([P, XCOLS], f16, tag="xs", name="xs")
                bch = CHUNKS[ch][0]
                eng = xq[q % 2]
                eng.dma_start(t[:, :4 * bch], xt_d[ch][q])
                if ch == 0:
                    # W quad rides right behind its x quad on the same ring
                    eng.dma_start(wres[q][:], w_d[q])
                prefetched[(ch, q)] = t

            # --- PE warmup: keep the PE busy through the HAM cold window
            # while the first x/W DMAs are in flight.  Garbage matmuls into
            # the slot chunk 0 will overwrite (start=True clears the bank).
            wtile = wmpool.tile([P, BBLK], f16, tag="wm", name="wm")
            nc.vector.memset(wtile[:], 0.0)
            wps = psumA.tile([P, BBLK], f32, tag="pa0", name="wps")

            x_issue(0, 0)
            x_issue(0, 1)

            for _ in range(7):
                nc.tensor.matmul(wps[:], wtile[:, 0:P], wtile[:],
                                 start=True, stop=True)

            n512 = 0
            for ch, (bch, nbb) in enumerate(CHUNKS):
                # PSUM: s=0 blocks in pool A, s=1 in pool B (nbb=2);
                # nbb=1 chunks alternate pools A, B, A, ...
                if nbb == 2:
                    pools = [(psumA, "a"), (psumB, "b")]
                else:
                    pools = [[(psumA, "a"), (psumB, "b")][n512 % 2]]
                    n512 += 1
                psums = [
                    [pool.tile([P, BBLK], f32, tag=f"p{pc}{us}",
                               name=f"ps{ch}_{us}_{s}")
                     for s, (pool, pc) in enumerate(pools)]
                    for us in range(NUS)
                ]

                for q in range(NXQ):
                    if (ch, q) not in prefetched:
                        x_issue(ch, q)
                    xs = prefetched.pop((ch, q))
                    for j in range(4):
                        kt = 4 * q + j
                        if kt >= KT:
                            break
                        # first k-tile after a boundary: touch the banks in
                        # the order the previous chunk's casts free them
                        us_order = [0, 2, 1, 3] if (kt == 0 and ch > 0) \
                            else range(NUS)
                        for us in us_order:
                            for s in range(nbb):
                                nc.tensor.matmul(
                                    psums[us][s][:],
                                    wres[q][:, j * U_SH + us * P:
                                            j * U_SH + (us + 1) * P],
                                    xs[:, j * bch + s * BBLK:
                                           j * bch + (s + 1) * BBLK],
                                    start=(kt == 0),
                                    stop=(kt == KT - 1),
                                )

                # Prefetch next chunk's first x quads BEFORE the drain --
                # the casts below block the engine streams on matmul sems.
                if ch + 1 < len(CHUNKS):
                    for qq in range(2):
                        x_issue(ch + 1, qq)

                # Drain: cast PSUM -> one stage tile; vector does us 0-1,
                # scalar does us 2-3 in parallel; pool-A banks (which the
                # next chunk needs first) are cast before pool-B banks.
                last = ch == len(CHUNKS) - 1
                st = spool.tile([P, NUS * nbb * BBLK], bf16,
                                tag="st0" if ch == 0 else "st_s",
                                name=f"st{ch}")
                for s in range(nbb):          # s=0 (pool A) first
                    for us in range(NUS):
                        dst = st[:, (us * nbb + s) * BBLK:
                                 (us * nbb + s + 1) * BBLK]
                        if us < 2:
                            nc.vector.tensor_copy(dst, psums[us][s][:])
                        else:
                            nc.scalar.copy(dst, psums[us][s][:])
                # out-DMA halves; non-final chunks ride the idle gpsimd
                # queue so they never block the x issue streams.
                half = 2 * nbb * BBLK
                oeng = (nc.sync, nc.scalar) if last else (nc.gpsimd, nc.gpsimd)
                oeng[0].dma_start(o_d[ch][:, 0:2], st[:, :half])
                oeng[1].dma_start(o_d[ch][:, 2:4], st[:, half:])

    nc.compile()
    return nc


def _get_nc():
    if "nc" not in _NC_CACHE:
        _NC_CACHE["nc"] = _build_nc()
    return _NC_CACHE["nc"]


def kernel(x, kernel_vector, bias, nonzero_ind):
    global LAST_RESULT
    from concourse.bass_utils import run_bass_kernel_spmd

    x = np.asarray(x, dtype=np.float32)
    kernel_vector = np.asarray(kernel_vector, dtype=np.float32)
    bias = np.asarray(bias, dtype=np.float32)
    nonzero_ind = np.asarray(nonzero_ind)

    nc = _get_nc()

    # Host scatter: dense weights [KPAD, U] fp16 (rows >= 20000 stay zero).
    rows = nonzero_ind[:, 0].astype(np.int64)
    cols = nonzero_ind[:, 1].astype(np.int64)
    w_full = np.zeros(KPAD * U, np.float32)
    np.add.at(w_full, rows * U + cols, kernel_vector)
    w_full = w_full.reshape(KPAD, U).astype(np.float16)

    # x^T padded to KPAD rows, fp16.
    x16 = x.astype(np.float16)
    xt = np.zeros((KPAD, B), np.float16)
    xt[:K] = x16.T

    # Per-chunk quad layout: xt{ch}[q, p, j*bch + b] = xt[(4q+j)*128+p, b0+b]
    xt_chunks = []
    b0 = 0
    for bch, nbb in CHUNKS:
        xc = xt[:, b0:b0 + bch].reshape(NXQ, 4, P, bch).transpose(0, 2, 1, 3)
        xt_chunks.append(np.ascontiguousarray(xc.reshape(NXQ, P, 4 * bch)))
        b0 += bch

    in_maps = []
    for c in range(USPLIT):
        # W quads: w[q, p, j*512 + u] = W[(4q+j)*128 + p, c*512 + u]
        w_sh = w_full[:, c * U_SH:(c + 1) * U_SH]
        w_sh = w_sh.reshape(NXQ, 4, P, U_SH).transpose(0, 2, 1, 3)
        w_sh = np.ascontiguousarray(w_sh.reshape(NXQ, P, 4 * U_SH))
        m = {"w_sh": w_sh}
        for ch in range(len(CHUNKS)):
            m[f"xt{ch}"] = xt_chunks[ch]
        in_maps.append(m)

    kwargs = {}
    if TRACE:
        kwargs = dict(trace=True, trace_cores=list(range(8)))
    res = run_bass_kernel_spmd(nc, in_maps, core_ids=list(range(8)), **kwargs)
    LAST_RESULT = res

    out = np.empty((B, U), np.float32)
    for c in range(USPLIT):
        b0 = 0
        for ch, (bch, nbb) in enumerate(CHUNKS):
            # [P, NUS, nbb*BBLK] -> [nbb, BBLK, NUS, P] -> [bch, U_SH]
            blk = (
                res.results[c][f"o{ch}"]
                .astype(np.float32)
                .reshape(P, NUS, nbb, BBLK)
                .transpose(2, 3, 1, 0)
                .reshape(bch, U_SH)
            )
            out[b0:b0 + bch, c * U_SH:(c + 1) * U_SH] = blk
            b0 += bch
    out += bias[None, :]
    np.tanh(out, out=out)
    return out


# revision 20
# speedup vs baseline: 1.0618x; 1.0416x over previous
"""Trainium2 kernel for: out = tanh(x @ scatter_nd(nonzero_ind, kernel_vector, (20000, 4096)) + bias).

Strategy (8 NeuronCores), W-resident / x-streaming, units sharded x8:
  core c owns W[:, c*512:(c+1)*512] (20096 x 512 fp16, SBUF-resident) and
  computes out[:, c*512:(c+1)*512] = x @ W_c for the full batch.

v6 = the baseline's PROVEN DMA steady-state (x stream: 256 KB tiles on
the gpsimd SWDGE queue at ~152 GB/s; W: 128 KB per-k-tile transfers
free-running on the sync/scalar HWDGE rings during chunk 0) plus
targeted startup/boundary/tail fixes that leave it untouched:
  - PE warmup: memset + 8 garbage matmuls at t=0 cover the first-DMA
    wait and the HAM cold-clock window (the PE otherwise starts at
    1.2 GHz and pays ~50% on everything in the first ~3.4us).
  - The first 4 x tiles go on the sync/scalar rings ahead of the W
    flood (ring FIFO: first issued = first done), so the first real
    matmul starts ~2.5us earlier than the SWDGE path allows.
  - Chunks [1024, 512, 512]: PSUM pool A holds the s=0 banks, pool B
    s=1; the 512-chunks alternate pools, so each boundary waits only
    for the first bank set's casts (~0.7us) and the c1->c2 boundary is
    free.  Casts split across Vector AND Scalar engines, A banks first;
    each chunk's four [128,512] results are cast into ONE stage tile
    and leave as two half DMAs -> tail ~4us instead of ~15us.
  - The next chunk's first x pairs are issued on gpsimd BEFORE the
    drain casts (whose matmul sem-waits block the sync/scalar streams).
  - x for the 512-chunks rides as k-tile PAIRS (256 KB -- the transfer
    size the SWDGE queue demonstrably sustains at ~152 GB/s).
"""

import numpy as np

P = 128
B, K, U = 2048, 20000, 4096
USPLIT = 8
KT = 157                 # k-tiles (full contraction per core)
KTP = 158                # padded to even for k-tile pairs
KPAD = KTP * P           # 20224 rows (224 zero pad)
U_SH = U // USPLIT       # 512 unit cols per core
NUS = U_SH // P          # 4 W subtiles (stationary blocks) per k-tile
NXP = KTP // 2           # 79 k-tile pairs for the 512-chunks

# chunk config: list of (batch_size, n_batch_blocks); BBLK = size // nbb = 512
CHUNKS = [(1024, 2), (512, 1), (512, 1)]
BBLK = 512

TRACE = False            # set by test harness for profiled runs
LAST_RESULT = None       # BassKernelResults of the last run (for the harness)

_NC_CACHE = {}


def _build_nc():
    from concourse import bacc
    import concourse.mybir as mybir
    import concourse.tile as tile

    f32 = mybir.dt.float32
    f16 = mybir.dt.float16
    bf16 = mybir.dt.bfloat16

    nc = bacc.Bacc("TRN2", target_bir_lowering=False, debug=False)

    # chunk 0 x tiles: xt0[kt, p, s*512 + b] = x[b, kt*128 + p] (fp16)
    # 512-chunk x pairs: xt{ch}[pair, p, j*512 + b] = x[b0 + b, (2pair+j)*128 + p]
    xt_d = []
    for ch, (bch, nbb) in enumerate(CHUNKS):
        if ch == 0:
            xt_d.append(nc.dram_tensor("xt0", [KT, P, bch], f16,
                                       kind="ExternalInput").ap())
        else:
            xt_d.append(nc.dram_tensor(f"xt{ch}", [NXP, P, 2 * bch], f16,
                                       kind="ExternalInput").ap())
    # W per k-tile: w[kt, p, u] = W[kt*128 + p, u]
    w_d = nc.dram_tensor("w_sh", [KT, P, U_SH], f16, kind="ExternalInput").ap()
    # out per chunk: o{ch}[p, us, s*BBLK + b] = z^T[us*128+p, s*BBLK+b]
    o_d = [
        nc.dram_tensor(f"o{ch}", [P, NUS, nbb * BBLK], bf16,
                       kind="ExternalOutput").ap()
        for ch, (bch, nbb) in enumerate(CHUNKS)
    ]

    with tile.TileContext(nc) as tc:
        with (
            tc.tile_pool(name="resid", bufs=1) as respool,
            tc.tile_pool(name="xpool", bufs=8) as xpool,
            tc.tile_pool(name="stage", bufs=1) as spool,
            tc.tile_pool(name="warm", bufs=1) as wmpool,
            tc.tile_pool(name="psumA", bufs=1, space="PSUM") as psumA,
            tc.tile_pool(name="psumB", bufs=1, space="PSUM") as psumB,
        ):
            prefetched = {}

            def x_issue(ch, seg, eng=None):
                # ch 0: seg = k-tile index; ch>0: seg = k-tile pair index
                t = xpool.tile([P, 1024], f16, tag="xs", name="xs")
                eng = eng or nc.gpsimd
                eng.dma_start(t[:], xt_d[ch][seg])
                prefetched[(ch, seg)] = t

            # --- PE warmup: keep the PE busy through the HAM cold window
            # while the first x/W DMAs are in flight.
            wtile = wmpool.tile([P, BBLK], f16, tag="wm", name="wm")
            nc.vector.memset(wtile[:], 0.0)
            wps = psumA.tile([P, BBLK], f32, tag="pa0", name="wps")

            # first 4 x tiles ride the HWDGE rings ahead of the W flood
            x_issue(0, 0, nc.sync)
            x_issue(0, 1, nc.scalar)
            x_issue(0, 2, nc.sync)
            x_issue(0, 3, nc.scalar)

            for _ in range(8):
                nc.tensor.matmul(wps[:], wtile[:, 0:P], wtile[:],
                                 start=True, stop=True)

            wres = [
                respool.tile([P, U_SH], f16, tag=f"w{kt}", name=f"w{kt}")
                for kt in range(KT)
            ]

            n512 = 0
            for ch, (bch, nbb) in enumerate(CHUNKS):
                # PSUM: s=0 blocks in pool A, s=1 in pool B (nbb=2);
                # nbb=1 chunks alternate pools A, B, A, ...
                if nbb == 2:
                    pools = [(psumA, "a"), (psumB, "b")]
                else:
                    pools = [[(psumA, "a"), (psumB, "b")][n512 % 2]]
                    n512 += 1
                psums = [
                    [pool.tile([P, BBLK], f32, tag=f"p{pc}{us}",
                               name=f"ps{ch}_{us}_{s}")
                     for s, (pool, pc) in enumerate(pools)]
                    for us in range(NUS)
                ]

                for kt in range(KT):
                    if ch == 0:
                        # W free-runs on the HWDGE rings (proven ~115 GB/s
                        # combined; fully resident by ~180us)
                        weng = nc.sync if kt % 2 == 0 else nc.scalar
                        weng.dma_start(wres[kt][:], w_d[kt])
                        seg, col0 = kt, 0
                    else:
                        seg, col0 = kt // 2, (kt % 2) * BBLK
                    if (ch, seg) not in prefetched:
                        x_issue(ch, seg)
                    xs = prefetched[(ch, seg)]
                    if ch == 0 or kt % 2 == 1 or kt == KT - 1:
                        del prefetched[(ch, seg)]  # last use of this tile
                    # first k-tile after a boundary: touch the banks in
                    # the order the previous chunk's casts free them
                    us_order = [0, 2, 1, 3] if (kt == 0 and ch > 0) \
                        else range(NUS)
                    for us in us_order:
                        for s in range(nbb):
                            nc.tensor.matmul(
                                psums[us][s][:],
                                wres[kt][:, us * P:(us + 1) * P],
                                xs[:, col0 + s * BBLK:
                                       col0 + (s + 1) * BBLK],
                                start=(kt == 0),
                                stop=(kt == KT - 1),
                            )

                # Prefetch next chunk's first x pairs on gpsimd BEFORE the
                # drain -- the casts block sync/scalar on matmul sems, and
                # the gpsimd stream has no such waits.
                if ch + 1 < len(CHUNKS):
                    for pp in range(3):
                        x_issue(ch + 1, pp)

                # Drain: cast PSUM -> one stage tile; vector does us 0-1,
                # scalar does us 2-3 in parallel; pool-A banks (which the
                # next chunk needs first) are cast before pool-B banks.
                st = spool.tile([P, NUS * nbb * BBLK], bf16,
                                tag="st0" if ch == 0 else "st_s",
                                name=f"st{ch}")
                for s in range(nbb):          # s=0 (pool A) first
                    for us in range(NUS):
                        dst = st[:, (us * nbb + s) * BBLK:
                                 (us * nbb + s + 1) * BBLK]
                        if us < 2:
                            nc.vector.tensor_copy(dst, psums[us][s][:])
                        else:
                            nc.scalar.copy(dst, psums[us][s][:])
                half = 2 * nbb * BBLK
                nc.sync.dma_start(o_d[ch][:, 0:2], st[:, :half])
                nc.scalar.dma_start(o_d[ch][:, 2:4], st[:, half:])

    nc.compile()
    return nc


def _get_nc():
    if "nc" not in _NC_CACHE:
        _NC_CACHE["nc"] = _build_nc()
    return _NC_CACHE["nc"]


def kernel(x, kernel_vector, bias, nonzero_ind):
    global LAST_RESULT
    from concourse.bass_utils import run_bass_kernel_spmd

    x = np.asarray(x, dtype=np.float32)
    kernel_vector = np.asarray(kernel_vector, dtype=np.float32)
    bias = np.asarray(bias, dtype=np.float32)
    nonzero_ind = np.asarray(nonzero_ind)

    nc = _get_nc()

    # Host scatter: dense weights [KT*P, U] fp16 (rows >= 20000 stay zero).
    rows = nonzero_ind[:, 0].astype(np.int64)
    cols = nonzero_ind[:, 1].astype(np.int64)
    w_full = np.zeros(KT * P * U, np.float32)
    np.add.at(w_full, rows * U + cols, kernel_vector)
    w_full = w_full.reshape(KT * P, U).astype(np.float16)

    # x^T padded to KPAD rows, fp16.
    x16 = x.astype(np.float16)
    xt = np.zeros((KPAD, B), np.float16)
    xt[:K] = x16.T

    xt_chunks = []
    b0 = 0
    for ch, (bch, nbb) in enumerate(CHUNKS):
        if ch == 0:
            xc = xt[:KT * P, b0:b0 + bch].reshape(KT, P, bch)
            xt_chunks.append(np.ascontiguousarray(xc))
        else:
            xc = (xt[:, b0:b0 + bch].reshape(NXP, 2, P, bch)
                  .transpose(0, 2, 1, 3))
            xt_chunks.append(np.ascontiguousarray(xc.reshape(NXP, P, 2 * bch)))
        b0 += bch

    in_maps = []
    for c in range(USPLIT):
        w_sh = np.ascontiguousarray(
            w_full[:, c * U_SH:(c + 1) * U_SH]).reshape(KT, P, U_SH)
        m = {"w_sh": w_sh}
        for ch in range(len(CHUNKS)):
            m[f"xt{ch}"] = xt_chunks[ch]
        in_maps.append(m)

    kwargs = {}
    if TRACE:
        kwargs = dict(trace=True, trace_cores=list(range(8)))
    res = run_bass_kernel_spmd(nc, in_maps, core_ids=list(range(8)), **kwargs)
    LAST_RESULT = res

    out = np.empty((B, U), np.float32)
    for c in range(USPLIT):
        b0 = 0
        for ch, (bch, nbb) in enumerate(CHUNKS):
            # [P, NUS, nbb*BBLK] -> [nbb, BBLK, NUS, P] -> [bch, U_SH]
            blk = (
                res.results[c][f"o{ch}"]
                .astype(np.float32)
                .reshape(P, NUS, nbb, BBLK)
                .transpose(2, 3, 1, 0)
                .reshape(bch, U_SH)
            )
            out[b0:b0 + bch, c * U_SH:(c + 1) * U_SH] = blk
            b0 += bch
    out += bias[None, :]
    np.tanh(out, out=out)
    return out


# revision 25
# speedup vs baseline: 1.0637x; 1.0018x over previous
"""Trainium2 kernel for: out = tanh(x @ scatter_nd(nonzero_ind, kernel_vector, (20000, 4096)) + bias).

Strategy (8 NeuronCores), W-resident / x-streaming, units sharded x8:
  core c owns W[:, c*512:(c+1)*512] (20096 x 512 fp16, SBUF-resident) and
  computes out[:, c*512:(c+1)*512] = x @ W_c for the full batch.

v6 = the baseline's PROVEN DMA steady-state (x stream: 256 KB tiles on
the gpsimd SWDGE queue at ~152 GB/s; W: 128 KB per-k-tile transfers
free-running on the sync/scalar HWDGE rings during chunk 0) plus
targeted startup/boundary/tail fixes that leave it untouched:
  - PE warmup: memset + 8 garbage matmuls at t=0 cover the first-DMA
    wait and the HAM cold-clock window (the PE otherwise starts at
    1.2 GHz and pays ~50% on everything in the first ~3.4us).
  - The first 4 x tiles go on the sync/scalar rings ahead of the W
    flood (ring FIFO: first issued = first done), so the first real
    matmul starts ~2.5us earlier than the SWDGE path allows.
  - Chunks [1024, 512, 512]: PSUM pool A holds the s=0 banks, pool B
    s=1; the 512-chunks alternate pools, so each boundary waits only
    for the first bank set's casts (~0.7us) and the c1->c2 boundary is
    free.  Casts split across Vector AND Scalar engines, A banks first;
    each chunk's four [128,512] results are cast into ONE stage tile
    and leave as two half DMAs -> tail ~4us instead of ~15us.
  - The next chunk's first x pairs are issued on gpsimd BEFORE the
    drain casts (whose matmul sem-waits block the sync/scalar streams).
  - x for the 512-chunks rides as k-tile PAIRS (256 KB -- the transfer
    size the SWDGE queue demonstrably sustains at ~152 GB/s).
"""

import numpy as np

P = 128
B, K, U = 2048, 20000, 4096
USPLIT = 8
KT = 157                 # k-tiles (full contraction per core)
KTP = 158                # padded to even for k-tile pairs
KPAD = KTP * P           # 20224 rows (224 zero pad)
U_SH = U // USPLIT       # 512 unit cols per core
NUS = U_SH // P          # 4 W subtiles (stationary blocks) per k-tile
NXP = KTP // 2           # 79 k-tile pairs for the 512-chunks

# chunk config: list of (batch_size, n_batch_blocks); BBLK = size // nbb = 512
CHUNKS = [(1024, 2), (512, 1), (512, 1)]
BBLK = 512

TRACE = False            # set by test harness for profiled runs
LAST_RESULT = None       # BassKernelResults of the last run (for the harness)

_NC_CACHE = {}


def _build_nc():
    from concourse import bacc
    import concourse.mybir as mybir
    import concourse.tile as tile

    f32 = mybir.dt.float32
    f16 = mybir.dt.float16
    bf16 = mybir.dt.bfloat16

    nc = bacc.Bacc("TRN2", target_bir_lowering=False, debug=False)

    # chunk 0 x tiles: xt0[kt, p, s*512 + b] = x[b, kt*128 + p] (fp16)
    # 512-chunk x pairs: xt{ch}[pair, p, j*512 + b] = x[b0 + b, (2pair+j)*128 + p]
    xt_d = []
    for ch, (bch, nbb) in enumerate(CHUNKS):
        if ch == 0:
            xt_d.append(nc.dram_tensor("xt0", [KT, P, bch], f16,
                                       kind="ExternalInput").ap())
        else:
            xt_d.append(nc.dram_tensor(f"xt{ch}", [NXP, P, 2 * bch], f16,
                                       kind="ExternalInput").ap())
    # W head: k-tiles 0-15 as four 4-k-tile quads (the per-k-tile trickle
    # is lane-paced at ~1.1us/k-tile -- too slow for the first k-tiles);
    # W tail: per k-tile 16..156.
    wh_d = nc.dram_tensor("w_head", [4, P, 4 * U_SH], f16,
                          kind="ExternalInput").ap()
    w_d = nc.dram_tensor("w_sh", [KT - 16, P, U_SH], f16,
                         kind="ExternalInput").ap()
    # out per chunk: o{ch}[p, us, s*BBLK + b] = z^T[us*128+p, s*BBLK+b]
    o_d = [
        nc.dram_tensor(f"o{ch}", [P, NUS, nbb * BBLK], bf16,
                       kind="ExternalOutput").ap()
        for ch, (bch, nbb) in enumerate(CHUNKS)
    ]

    with tile.TileContext(nc) as tc:
        with (
            tc.tile_pool(name="resid", bufs=1) as respool,
            tc.tile_pool(name="xpool", bufs=8) as xpool,
            tc.tile_pool(name="stage", bufs=1) as spool,
            tc.tile_pool(name="warm", bufs=1) as wmpool,
            tc.tile_pool(name="psumA", bufs=1, space="PSUM") as psumA,
            tc.tile_pool(name="psumB", bufs=1, space="PSUM") as psumB,
        ):
            prefetched = {}

            def x_issue(ch, seg, eng=None):
                # ch 0: seg = k-tile index; ch>0: seg = k-tile pair index
                t = xpool.tile([P, 1024], f16, tag="xs", name="xs")
                eng = eng or nc.gpsimd
                eng.dma_start(t[:], xt_d[ch][seg])
                prefetched[(ch, seg)] = t

            # --- PE warmup: keep the PE busy through the HAM cold window
            # while the first x/W DMAs are in flight.
            wtile = wmpool.tile([P, BBLK], f16, tag="wm", name="wm")
            nc.vector.memset(wtile[:], 0.0)
            wps = psumA.tile([P, BBLK], f32, tag="pa0", name="wps")

            whead = [
                respool.tile([P, 4 * U_SH], f16, tag=f"wh{g}", name=f"wh{g}")
                for g in range(4)
            ]
            wres = [
                respool.tile([P, U_SH], f16, tag=f"w{kt}", name=f"w{kt}")
                for kt in range(16, KT)
            ]

            # first 4 x tiles ride the HWDGE rings ahead of the W flood,
            # then the W head quads (k-tiles 0-15).
            x_issue(0, 0, nc.sync)
            x_issue(0, 1, nc.scalar)
            x_issue(0, 2, nc.sync)
            x_issue(0, 3, nc.scalar)
            for g in range(4):
                (nc.sync if g % 2 == 0 else nc.scalar).dma_start(
                    whead[g][:], wh_d[g])

            for _ in range(8):
                nc.tensor.matmul(wps[:], wtile[:, 0:P], wtile[:],
                                 start=True, stop=True)

            n512 = 0
            for ch, (bch, nbb) in enumerate(CHUNKS):
                # PSUM: s=0 blocks in pool A, s=1 in pool B (nbb=2);
                # nbb=1 chunks alternate pools A, B, A, ...
                if nbb == 2:
                    pools = [(psumA, "a"), (psumB, "b")]
                else:
                    pools = [[(psumA, "a"), (psumB, "b")][n512 % 2]]
                    n512 += 1
                psums = [
                    [pool.tile([P, BBLK], f32, tag=f"p{pc}{us}",
                               name=f"ps{ch}_{us}_{s}")
                     for s, (pool, pc) in enumerate(pools)]
                    for us in range(NUS)
                ]

                for kt in range(KT):
                    if ch == 0:
                        if kt >= 16:
                            # W tail free-runs on the HWDGE rings (proven
                            # ~115 GB/s combined; fully resident by ~180us)
                            weng = nc.sync if kt % 2 == 0 else nc.scalar
                            weng.dma_start(wres[kt - 16][:], w_d[kt - 16])
                        seg, col0 = kt, 0
                    else:
                        seg, col0 = kt // 2, (kt % 2) * BBLK
                    if (ch, seg) not in prefetched:
                        x_issue(ch, seg)
                    xs = prefetched[(ch, seg)]
                    if ch == 0 or kt % 2 == 1 or kt == KT - 1:
                        del prefetched[(ch, seg)]  # last use of this tile
                    # first k-tile after a boundary: touch the banks in
                    # the order the previous chunk's casts free them
                    us_order = [0, 2, 1, 3] if (kt == 0 and ch > 0) \
                        else range(NUS)
                    if kt < 16:
                        wsl = whead[kt // 4][:, (kt % 4) * U_SH:
                                             (kt % 4 + 1) * U_SH]
                    else:
                        wsl = wres[kt - 16][:]
                    for us in us_order:
                        for s in range(nbb):
                            nc.tensor.matmul(
                                psums[us][s][:],
                                wsl[:, us * P:(us + 1) * P],
                                xs[:, col0 + s * BBLK:
                                       col0 + (s + 1) * BBLK],
                                start=(kt == 0),
                                stop=(kt == KT - 1),
                            )

                # Prefetch next chunk's first x pairs on gpsimd BEFORE the
                # drain -- the casts block sync/scalar on matmul sems, and
                # the gpsimd stream has no such waits.
                if ch + 1 < len(CHUNKS):
                    for pp in range(3):
                        x_issue(ch + 1, pp)

                # Drain: cast PSUM -> one stage tile; vector does us 0-1,
                # scalar does us 2-3 in parallel; pool-A banks (which the
                # next chunk needs first) are cast before pool-B banks.
                st = spool.tile([P, NUS * nbb * BBLK], bf16,
                                tag="st0" if ch == 0 else "st_s",
                                name=f"st{ch}")
                for s in range(nbb):          # s=0 (pool A) first
                    for us in range(NUS):
                        dst = st[:, (us * nbb + s) * BBLK:
                                 (us * nbb + s + 1) * BBLK]
                        if us < 2:
                            nc.vector.tensor_copy(dst, psums[us][s][:])
                        else:
                            nc.scalar.copy(dst, psums[us][s][:])
                half = 2 * nbb * BBLK
                nc.sync.dma_start(o_d[ch][:, 0:2], st[:, :half])
                nc.scalar.dma_start(o_d[ch][:, 2:4], st[:, half:])

    nc.compile()
    return nc


def _get_nc():
    if "nc" not in _NC_CACHE:
        _NC_CACHE["nc"] = _build_nc()
    return _NC_CACHE["nc"]


def kernel(x, kernel_vector, bias, nonzero_ind):
    global LAST_RESULT
    from concourse.bass_utils import run_bass_kernel_spmd

    x = np.asarray(x, dtype=np.float32)
    kernel_vector = np.asarray(kernel_vector, dtype=np.float32)
    bias = np.asarray(bias, dtype=np.float32)
    nonzero_ind = np.asarray(nonzero_ind)

    nc = _get_nc()

    # Host scatter: dense weights [KT*P, U] fp16 (rows >= 20000 stay zero).
    rows = nonzero_ind[:, 0].astype(np.int64)
    cols = nonzero_ind[:, 1].astype(np.int64)
    w_full = np.zeros(KT * P * U, np.float32)
    np.add.at(w_full, rows * U + cols, kernel_vector)
    w_full = w_full.reshape(KT * P, U).astype(np.float16)

    # x^T padded to KPAD rows, fp16.
    x16 = x.astype(np.float16)
    xt = np.zeros((KPAD, B), np.float16)
    xt[:K] = x16.T

    xt_chunks = []
    b0 = 0
    for ch, (bch, nbb) in enumerate(CHUNKS):
        if ch == 0:
            xc = xt[:KT * P, b0:b0 + bch].reshape(KT, P, bch)
            xt_chunks.append(np.ascontiguousarray(xc))
        else:
            xc = (xt[:, b0:b0 + bch].reshape(NXP, 2, P, bch)
                  .transpose(0, 2, 1, 3))
            xt_chunks.append(np.ascontiguousarray(xc.reshape(NXP, P, 2 * bch)))
        b0 += bch

    in_maps = []
    for c in range(USPLIT):
        wc = w_full[:, c * U_SH:(c + 1) * U_SH]
        w_head = np.ascontiguousarray(
            wc[:16 * P].reshape(4, 4, P, U_SH).transpose(0, 2, 1, 3)
            .reshape(4, P, 4 * U_SH))
        w_sh = np.ascontiguousarray(
            wc[16 * P:].reshape(KT - 16, P, U_SH))
        m = {"w_sh": w_sh, "w_head": w_head}
        for ch in range(len(CHUNKS)):
            m[f"xt{ch}"] = xt_chunks[ch]
        in_maps.append(m)

    kwargs = {}
    if TRACE:
        kwargs = dict(trace=True, trace_cores=list(range(8)))
    res = run_bass_kernel_spmd(nc, in_maps, core_ids=list(range(8)), **kwargs)
    LAST_RESULT = res

    out = np.empty((B, U), np.float32)
    for c in range(USPLIT):
        b0 = 0
        for ch, (bch, nbb) in enumerate(CHUNKS):
            # [P, NUS, nbb*BBLK] -> [nbb, BBLK, NUS, P] -> [bch, U_SH]
            blk = (
                res.results[c][f"o{ch}"]
                .astype(np.float32)
                .reshape(P, NUS, nbb, BBLK)
                .transpose(2, 3, 1, 0)
                .reshape(bch, U_SH)
            )
            out[b0:b0 + bch, c * U_SH:(c + 1) * U_SH] = blk
            b0 += bch
    out += bias[None, :]
    np.tanh(out, out=out)
    return out
